# revision 1
# baseline (speedup 1.0000x reference)
"""Performer (FAVOR+) causal linear attention on 8 Trainium2 NeuronCores.

Problem: q,k,v [2,16,4096,64] f32, proj [64,64], chunk=128, causal chunked
linear attention with positive softmax features (see reference).

Sharding: data-parallel over b*h = 32 heads -> 4 heads per core, no
collectives. Each core runs an identical Bass program on its 4 heads.

Math (validated in proto.py against the jax reference, rel err ~1e-6):
  dn = d**-0.25, ratio = m**-0.5
  dd      = (x @ (proj*dn))            [L, M]   ("data_dash", no diag)
  diag    = 0.0625 * sum(x*x, -1)      [L, 1]
  stab_q  = max_m dd                   per token;  stab_k = global max
  feat    = exp(dd - diag - stab + ln(ratio)) + ratio*EPS
  attention: per 128-chunk c:
    scoresT = (kp_c @ qp_c^T) * maskT   (maskT[j,i] = j<=i)
    out_c   = scoresT^T @ [v_c|1] + qp_c @ S     (S = running sum kp^T [v|1])
    o_c     = out_c[:, :64] / out_c[:, 64]
"""
import math
import os
from contextlib import ExitStack

import numpy as np
import ml_dtypes

import concourse.bass as bass
import concourse.bacc as bacc
import concourse.tile as tile
from concourse import mybir
import concourse.bass_isa as bass_isa
from concourse.bass import ts
from concourse.bass_utils import run_bass_kernel_spmd

F32 = mybir.dt.float32
F32R = mybir.dt.float32r
BF16 = mybir.dt.bfloat16

B, H, L, D, M = 2, 16, 4096, 64, 64
NCORES = 8
HPC = (B * H) // NCORES          # heads per core = 4
CHUNK = 128
NCH = L // CHUNK                 # 32 chunks
TIL = 512
NT = L // TIL                    # 8 tiles
CPT = TIL // CHUNK               # 4 chunks per tile

DN = D ** -0.25
RATIO = M ** -0.5
LN_RATIO = math.log(RATIO)
NDIAG = -0.5 * DN * DN           # -0.0625
REPS = RATIO * 1e-4

# knobs
TR_IN_DT = F32      # dtype mode for input transposes (f32r: 1.5 cyc/row)
DD_DT = F32         # dtype for the feature matmul (precision-critical)
ADD = mybir.AluOpType.add
SUB = mybir.AluOpType.subtract
MULT = mybir.AluOpType.mult
MAXOP = mybir.AluOpType.max
AXX = mybir.AxisListType.X
EXP = mybir.ActivationFunctionType.Exp
COPYF = mybir.ActivationFunctionType.Copy


def _bc(ap, n, pos):
    """broadcast AP: insert [0, n] at free-dim position pos (1-based in ap list)."""
    return bass.AP(tensor=ap.tensor, offset=ap.offset,
                   ap=list(ap.ap[:pos]) + [[0, n]] + list(ap.ap[pos:]))


def build_program():
    nc = bacc.Bacc("TRN2", target_bir_lowering=False, debug=False)
    q = nc.dram_tensor("q", [HPC, L, D], F32, kind="ExternalInput")
    k = nc.dram_tensor("k", [HPC, L, D], F32, kind="ExternalInput")
    v = nc.dram_tensor("v", [HPC, L, D], F32, kind="ExternalInput")
    proj_s = nc.dram_tensor("proj_s", [D, M], F32, kind="ExternalInput")
    id32 = nc.dram_tensor("id32", [128, 128], F32, kind="ExternalInput")
    id16 = nc.dram_tensor("id16", [128, 128], BF16, kind="ExternalInput")
    maskt = nc.dram_tensor("maskt", [CHUNK, CHUNK], BF16, kind="ExternalInput")
    o = nc.dram_tensor("o", [HPC, L, D], F32, kind="ExternalOutput")

    with ExitStack() as ctx:
        tc = ctx.enter_context(tile.TileContext(nc))
        consts = ctx.enter_context(tc.tile_pool(name="consts", bufs=1))
        p_head = ctx.enter_context(tc.tile_pool(name="head", bufs=2))
        p_small = ctx.enter_context(tc.tile_pool(name="small", bufs=3))
        p_xin = ctx.enter_context(tc.tile_pool(name="xin", bufs=4))
        p_xT = ctx.enter_context(tc.tile_pool(name="xT", bufs=3))
        p_scr = ctx.enter_context(tc.tile_pool(name="scr", bufs=6))
        p_ssb = ctx.enter_context(tc.tile_pool(name="ssb", bufs=6))
        p_osb = ctx.enter_context(tc.tile_pool(name="osb", bufs=4))
        ps_big = ctx.enter_context(tc.tile_pool(name="psbig", bufs=3, space="PSUM"))
        ps_med = ctx.enter_context(tc.tile_pool(name="psmed", bufs=3, space="PSUM"))
        ps_s = ctx.enter_context(tc.tile_pool(name="pss", bufs=1, space="PSUM"))

        c_proj = consts.tile([D, M], F32)
        nc.sync.dma_start(out=c_proj, in_=proj_s[:, :])
        c_id32 = consts.tile([128, 128], F32)
        nc.sync.dma_start(out=c_id32, in_=id32[:, :])
        c_id16 = consts.tile([128, 128], BF16)
        nc.sync.dma_start(out=c_id16, in_=id16[:, :])
        c_mask = consts.tile([CHUNK, CHUNK], BF16)
        nc.sync.dma_start(out=c_mask, in_=maskt[:, :])

        for h in range(int(os.environ.get("KERNEL_HEADS", str(HPC)))):
            build_head(nc, tc, h, q, k, v, o,
                       c_proj, c_id32, c_id16, c_mask,
                       p_head, p_small, p_xin, p_xT, p_scr, p_ssb, p_osb,
                       ps_big, ps_med, ps_s)
    nc.compile()
    return nc


def feat_tile(nc, x, h, t, pools, dest_T=None, dest_nat_dram=None,
              ssq=None, stab=None, biasq=None, out_kind='q',
              qp_pool=None):
    """One 512-token tile of the feature pipeline, phase 1 (shared q/k)."""
    (p_xin, p_xT, ps_big, ps_med) = pools
    # one DMA per 1024-token pair of tiles (halves per-DMA fixed cost);
    # even t allocates and loads 2 tiles, odd t reuses the second half
    if t % 2 == 0:
        x_pair = p_xin.tile([128, 2, CPT, D], F32, tag="x_nat")
        nc.sync.dma_start(
            out=x_pair,
            in_=x[h, ts(t // 2, 2 * TIL), :].rearrange(
                "(c p) d -> p c d", p=128).rearrange(
                "p (u c) d -> p u c d", u=2))
        nc._x_pair = x_pair
    x_nat = nc._x_pair[:, t % 2, :, :]
    # sum of squares per token (for diag): gpsimd square + DVE reduce
    scrap = p_xin.tile([128, CPT, D], F32, tag="scrap")
    nc.gpsimd.tensor_tensor(out=scrap, in0=x_nat, in1=x_nat, op=MULT)
    nc.vector.reduce_sum(out=ssq[:, ts(t, CPT)], in_=scrap, axis=AXX)
    # transpose input chunks -> [64, 512] psum -> sbuf
    ps_tr = ps_big.tile([64, TIL], F32, tag="ps_big")
    for c in range(CPT):
        nc.tensor.transpose(ps_tr[:, ts(c, 128)].bitcast(TR_IN_DT),
                            x_nat[:, c, :].bitcast(TR_IN_DT),
                            nc._c_id32.bitcast(TR_IN_DT))
    xT = p_xT.tile([64, TIL], F32, tag="xT")
    if t % 3 != 2:
        nc.scalar.copy(out=xT, in_=ps_tr)
    else:
        nc.vector.tensor_copy(out=xT, in_=ps_tr)
    # feature matmul: dd_nat[l, m] = x @ proj_s   (lhsT = xT chunk)
    ps_dd = ps_med.tile([128, CPT, M], F32, tag="ps_med")
    for c in range(CPT):
        nc.tensor.matmul(ps_dd[:, c, :],
                         lhsT=xT[:, ts(c, 128)].bitcast(DD_DT),
                         rhs=nc._c_proj.bitcast(DD_DT),
                         start=True, stop=True)
    return x_nat, ps_dd


def build_head(nc, tc, h, q, k, v, o, c_proj, c_id32, c_id16, c_mask,
               p_head, p_small, p_xin, p_xT, p_scr, p_ssb, p_osb,
               ps_big, ps_med, ps_s):
    STAGE = int(os.environ.get("KERNEL_STAGE", "6"))
    nc._c_proj = c_proj
    nc._c_id32 = c_id32
    pools = (p_xin, p_xT, ps_big, ps_med)

    def dump(tile_ap, tok0):
        # debug: write [128, CPT, D]-shaped tile into o rows [tok0, tok0+512)
        nc.sync.dma_start(
            out=o[h, tok0:tok0 + TIL, :].rearrange("(c p) d -> p c d", p=128),
            in_=tile_ap)

    # ---------------- K features (two-pass: global stab) ----------------
    ssq_k = p_small.tile([128, NCH], F32, tag="ssq_k")
    stabk = p_small.tile([128, NCH], F32, tag="stabk")
    ddk = p_head.tile([128, NT, CPT, M], F32, tag="ddk")
    for t in range(NT):
        _, ps_dd = feat_tile(nc, k, h, t, pools, ssq=ssq_k)
        nc.scalar.copy(out=ddk[:, t, :, :], in_=ps_dd)
        # reduce from the SBUF copy (2x DVE rate vs 1x PSUM reads)
        nc.vector.reduce_max(out=stabk[:, ts(t, CPT)], in_=ddk[:, t, :, :],
                             axis=AXX)
    if STAGE <= 1:
        dump(ddk[:, 0, :, :], 0)
        return

    # ---------------- Q features (single pass, per-token stab) ----------------
    # placed between K pass-1 and pass-2 so the scheduler fills the global-stab
    # barrier with Q work
    ssq_q = p_small.tile([128, NCH], F32, tag="ssq_q")
    qpT = p_head.tile([64, L], BF16, tag="qpT")
    for t in range(NT):
        _, ps_dd = feat_tile(nc, q, h, t, pools, ssq=ssq_q)
        ncmax = p_small.tile([128, CPT], F32, tag="ncmax")
        nc.vector.reduce_max(out=ncmax, in_=ps_dd, axis=AXX, negate=True)
        # biasq = ncmax + (NDIAG*ssq + LN_RATIO)
        biasq = p_small.tile([128, CPT], F32, tag="biasq")
        nc.vector.tensor_scalar(out=biasq, in0=ssq_q[:, ts(t, CPT)],
                                scalar1=NDIAG, scalar2=LN_RATIO,
                                op0=MULT, op1=ADD)
        nc.vector.tensor_tensor(out=biasq, in0=biasq, in1=ncmax, op=ADD)
        # add bias into psum (broadcast along m), then exp -> bf16
        nc.vector.tensor_tensor(out=ps_dd, in0=ps_dd,
                                in1=_bc(biasq, M, 2), op=ADD)
        qp_nat = p_scr.tile([128, CPT, M], BF16, tag="qp_nat")
        nc.scalar.activation(out=qp_nat, in_=ps_dd, func=EXP)
        nc.gpsimd.tensor_scalar(out=qp_nat, in0=qp_nat, scalar1=REPS,
                                scalar2=None, op0=ADD)
        ps_ft = ps_big.tile([64, TIL], BF16, tag="ps_big")
        for c in range(CPT):
            nc.tensor.transpose(ps_ft[:, ts(c, 128)], qp_nat[:, c, :], c_id16)
        nc.vector.tensor_copy(out=qpT[:, ts(t, TIL)], in_=ps_ft)

    # ---------------- V load + cast ----------------
    v_f32 = p_head.tile([128, NCH, D], F32, tag="v_f32")
    nc.sync.dma_start(out=v_f32,
                      in_=v[h, :, :].rearrange("(c p) d -> p c d", p=128))
    v_ext = p_head.tile([128, NCH, D + 1], BF16, tag="v_ext")
    nc.gpsimd.tensor_copy(out=v_ext[:, :, 0:D], in_=v_f32)
    nc.gpsimd.memset(v_ext[:, :, D:D + 1], 1.0)

    # global stab: free-dim max -> cross-partition max (broadcast to all)
    s1 = p_small.tile([128, 1], F32, tag="s1")
    nc.vector.reduce_max(out=s1, in_=stabk, axis=AXX)
    skbc = p_small.tile([128, 1], F32, tag="skbc")
    nc.gpsimd.partition_all_reduce(skbc, s1, channels=128,
                                   reduce_op=bass_isa.ReduceOp.max)
    # biask[:, j] = LN_RATIO - skbc - 0.0625*ssq_k[:, j]
    biask = p_small.tile([128, NCH], F32, tag="biask")
    nc.vector.tensor_scalar(out=biask, in0=ssq_k, scalar1=NDIAG,
                            scalar2=LN_RATIO, op0=MULT, op1=ADD)
    nc.vector.tensor_scalar(out=biask, in0=biask, scalar1=skbc,
                            scalar2=None, op0=SUB)
    # pass 2: exp -> kp_nat (bf16) with per-tile eps, fused transpose -> kpT
    kp_nat = p_head.tile([128, NCH, M], BF16, tag="kp_nat")
    kpT = p_head.tile([64, L], BF16, tag="kpT")
    for t in range(NT):
        kdb = p_scr.tile([128, CPT, M], F32, tag="kdb")
        nc.gpsimd.tensor_tensor(out=kdb, in0=ddk[:, t, :, :],
                                in1=_bc(biask[:, ts(t, CPT)], M, 2), op=ADD)
        nc.scalar.activation(out=kp_nat[:, ts(t, CPT), :], in_=kdb, func=EXP)
        nc.gpsimd.tensor_scalar(out=kp_nat[:, ts(t, CPT), :],
                                in0=kp_nat[:, ts(t, CPT), :], scalar1=REPS,
                                scalar2=None, op0=ADD)
        ps_ft = ps_big.tile([64, TIL], BF16, tag="ps_big")
        for c in range(CPT):
            nc.tensor.transpose(ps_ft[:, ts(c, 128)],
                                kp_nat[:, t * CPT + c, :], c_id16)
        nc.scalar.copy(out=kpT[:, ts(t, TIL)], in_=ps_ft)
    if STAGE <= 4:
        dump(ddk[:, 1, :, :], 0)
        return

    # ---------------- attention ----------------
    # running KV state split into even/odd accumulators so the
    # PE->ACT(copy)->PE chain has 2 chunks of slack
    ps_S0 = ps_s.tile([64, D + 1], F32, tag="ps_S0")
    ps_S1 = ps_s.tile([64, D + 1], F32, tag="ps_S1")
    s_prev = [None, None]
    for g in range(NT):
        ps_sc = ps_big.tile([128, CPT, CHUNK], F32, tag="ps_big")
        for ci in range(CPT):
            c = g * CPT + ci
            nc.tensor.matmul(ps_sc[:, ci, :], lhsT=kpT[:, ts(c, CHUNK)],
                             rhs=qpT[:, ts(c, CHUNK)], start=True, stop=True)
        scT = p_ssb.tile([128, CPT, CHUNK], BF16, tag="scT")
        nc.vector.tensor_tensor(out=scT, in0=ps_sc,
                                in1=_bc(c_mask, CPT, 1), op=MULT)
        ps_out = ps_med.tile([128, CPT, D + 1], F32, tag="ps_med")
        for ci in range(CPT):
            c = g * CPT + ci
            n_inter = sum(1 for s in s_prev if s is not None) if STAGE > 5 else 0
            nc.tensor.matmul(ps_out[:, ci, :], lhsT=scT[:, ci, :],
                             rhs=v_ext[:, c, :], start=True,
                             stop=(n_inter == 0))
            done = 0
            for s in s_prev:
                if s is None or STAGE <= 5:
                    continue
                done += 1
                nc.tensor.matmul(ps_out[:, ci, :], lhsT=qpT[:, ts(c, CHUNK)],
                                 rhs=s, start=False, stop=(done == n_inter))
            if STAGE > 5:
                # running state update (exclusive prefix: used by chunk c+2)
                par = c % 2
                ps_S = ps_S0 if par == 0 else ps_S1
                nc.tensor.matmul(ps_S, lhsT=kp_nat[:, c, :], rhs=v_ext[:, c, :],
                                 start=(c == par), stop=(c >= NCH - 2),
                                 skip_group_check=True)
                s_new = p_ssb.tile([64, D + 1], BF16, tag="s_sb")
                nc.scalar.activation(out=s_new, in_=ps_S, func=COPYF)
                s_prev[par] = s_new
        rden = p_small.tile([128, CPT], F32, tag="rden")
        nc.vector.reciprocal(out=rden, in_=ps_out[:, :, D])
        o_sb = p_osb.tile([128, CPT, D], F32, tag="o_sb")
        nc.vector.tensor_tensor(out=o_sb, in0=ps_out[:, :, 0:D],
                                in1=_bc(rden, D, 2), op=MULT)
        nc.sync.dma_start(
            out=o[h, ts(g, TIL), :].rearrange("(c p) d -> p c d", p=128),
            in_=o_sb)


_prog_cache = {}


def _get_program():
    if "nc" not in _prog_cache:
        _prog_cache["nc"] = build_program()
    return _prog_cache["nc"]


def _host_consts():
    dn = np.float32(DN)
    eye32 = np.eye(128, dtype=np.float32)
    eye16 = np.eye(128, dtype=ml_dtypes.bfloat16)
    maskt = np.triu(np.ones((CHUNK, CHUNK), np.float32)).astype(ml_dtypes.bfloat16)
    return eye32, eye16, maskt


def kernel(q, k, v, projection_matrix, chunk_size):
    q = np.asarray(q, np.float32)
    k = np.asarray(k, np.float32)
    v = np.asarray(v, np.float32)
    proj = np.asarray(projection_matrix, np.float32)
    assert int(np.asarray(chunk_size)) == CHUNK
    nc = _get_program()
    proj_s = (proj * np.float32(DN)).astype(np.float32)
    eye32, eye16, maskt = _host_consts()
    qf = q.reshape(B * H, L, D)
    kf = k.reshape(B * H, L, D)
    vf = v.reshape(B * H, L, D)
    in_maps = []
    for i in range(NCORES):
        sl = slice(i * HPC, (i + 1) * HPC)
        in_maps.append(dict(q=np.ascontiguousarray(qf[sl]),
                            k=np.ascontiguousarray(kf[sl]),
                            v=np.ascontiguousarray(vf[sl]),
                            proj_s=proj_s, id32=eye32, id16=eye16,
                            maskt=maskt))
    trace = bool(int(os.environ.get("KERNEL_TRACE", "0")))
    res = run_bass_kernel_spmd(nc, in_maps, list(range(NCORES)), trace=trace)
    if trace and res.exec_time_ns is not None:
        print(f"HW exec time: {res.exec_time_ns} ns")
    out = np.stack([res.results[i]["o"] for i in range(NCORES)], axis=0)
    return out.reshape(B, H, L, D).astype(np.float32)


if __name__ == "__main__":
    # smoke test with random data
    rng = np.random.default_rng(0)
    q = rng.standard_normal((B, H, L, D), dtype=np.float32)
    k = rng.standard_normal((B, H, L, D), dtype=np.float32)
    v = rng.standard_normal((B, H, L, D), dtype=np.float32)
    p = rng.standard_normal((D, M), dtype=np.float32)
    out = kernel(q, k, v, p, 128)
    print("ok", out.shape, out.dtype, np.abs(out).max())

